# revision 1
# baseline (speedup 1.0000x reference)
import sys

if "/opt/trn_rl_repo" not in sys.path:
    sys.path.insert(0, "/opt/trn_rl_repo")

import numpy as np

N = 3_000_000
NCORES = 8
NPC = N // NCORES          # 375_000 samples per core
PART = 128                 # SBUF partitions
SPP = 2944                 # samples per partition (padded)
NPADPC = PART * SPP        # 376_832
ROW = SPP * 9              # elements per partition
NT = 2                     # tiles per core
K = SPP // NT              # 1472 samples per tile per partition

# fp16 everywhere (DVE tensor_tensor is 2x-pumped for 16-bit dtypes and the
# data is well-conditioned). Work is emitted as ~66 WIDE DVE instructions
# per tile over multi-plane views instead of ~135 single-plane ops, so the
# fixed per-instruction cost (SBUF access bubble + queue slack) amortizes.
# 1/I3 is exp(-ln(I3)) on ACT; ln/exp/square/copy share one table set
# (natural_log_exp_and_others) so there is a single table load.

SQRT02 = 0.4472135954999579  # sqrt(0.2)

_cache = {}


def _emit_tile(nc, sp, io, ps, mybir, fin_d, pout_d, eye, eye8, t, pre=None,
               mid_hook=None):
    f16 = mybir.dt.float16
    f32 = mybir.dt.float32
    AL = mybir.AluOpType
    AF = mybir.ActivationFunctionType
    TT = nc.vector.tensor_tensor
    ACT = nc.scalar.activation
    MM = nc.tensor.matmul
    P = PART

    _ptag = [0]

    def acc_psum(terms):
        """Sum [P,1,K] SBUF planes on the TensorEngine via identity-weight
        matmul accumulation in PSUM (512-col chunks = one bank per matmul).
        Each term is an AP or (AP, weight) with weight eye (1x) or eye8 (8x).
        Returns the fp32 PSUM accumulator [P, K]."""
        _ptag[0] ^= 1
        acc = ps.tile([P, K], f32, name=f"ps{_ptag[0]}", tag=f"ps{_ptag[0]}")
        flat = [(tm[0] if isinstance(tm, tuple) else tm,
                 tm[1] if isinstance(tm, tuple) else eye) for tm in terms]
        flat = [(pl.rearrange("p n k -> p (n k)"), w) for pl, w in flat]
        for lo in range(0, K, 512):
            hi = min(lo + 512, K)
            for i, (pl, w) in enumerate(flat):
                MM(acc[:, lo:hi], w, pl[:, lo:hi],
                   start=(i == 0), stop=(i == len(flat) - 1))
        return acc

    def acc_chain(terms, out_plane, scale=1.0):
        acc = acc_psum(terms)
        ACT(out_plane.rearrange("p n k -> p (n k)"), acc, AF.Copy, scale=scale)

    def tl(tag, n, pool=None, bufs=1):
        pl = pool or sp
        return pl.tile([P, n, K], f16, name=tag, tag=tag, bufs=bufs)

    def bc(plane, n):
        return plane.broadcast_to([P, n, K])

    # ---- input + squares: either fresh, or prefetched during the
    # previous tile's FS stage (pre = (FP, SQa, SQb, SQc) on C/D/GT tags)
    if pre is None:
        # head tile: stream the input per F-row so compute starts after the
        # first third of the DMA instead of the whole 3.4MB transfer
        FP = tl("FP", 9, pool=io, bufs=2)
        inv = fin_d.rearrange("p (n s) -> p n s", n=9, s=SPP)[
            :, :, t * K:(t + 1) * K]
        for r in range(3):
            nc.sync.dma_start(FP[:, 3 * r:3 * r + 3, :], inv[:, 3 * r:3 * r + 3, :])
        SQa = tl("A", 3)
        ACT(SQa, FP[:, 0:3, :], AF.Square)        # sf0 sf1 sf2
        SQb = tl("B", 3)
        ACT(SQb, FP[:, 3:6, :], AF.Square)        # sf3 sf4 sf5
        SQc = tl("C", 3)
        ACT(SQc, FP[:, 6:9, :], AF.Square)        # sf6 sf7 sf8
        pa, pb, pc, sqc_tag = "D", "E", "F", "D"
        P1a, P1b, P2 = tl(pa, 3), tl(pb, 3), tl(pc, 3)
        for r in range(3):  # row r products depend only on row-r planes
            TT(P1a[:, r:r + 1, :], FP[:, 3 * r:3 * r + 1, :],
               FP[:, 3 * r + 1:3 * r + 2, :], AL.mult)
            TT(P1b[:, r:r + 1, :], FP[:, 3 * r + 1:3 * r + 2, :],
               FP[:, 3 * r + 2:3 * r + 3, :], AL.mult)
            TT(P2[:, r:r + 1, :], FP[:, 3 * r:3 * r + 1, :],
               FP[:, 3 * r + 2:3 * r + 3, :], AL.mult)
    else:
        FP, SQa, SQb, SQc = pre
        pa, pb, pc, sqc_tag = "A", "B", "F", "D"
        P1a = tl(pa, 3)
        TT(P1a, FP[:, 0:9:3, :], FP[:, 1:9:3, :], AL.mult)   # f0f1 f3f4 f6f7
        P1b = tl(pb, 3)
        TT(P1b, FP[:, 1:9:3, :], FP[:, 2:9:3, :], AL.mult)   # f1f2 f4f5 f7f8
        P2 = tl(pc, 3)
        TT(P2, FP[:, 0:9:3, :], FP[:, 2:9:3, :], AL.mult)    # f0f2 f3f5 f6f8
    CO = tl("G", 3)                                # [c01 c02 c12]
    U = tl("u1", 1)
    TT(U, P1a[:, 0:1, :], P1a[:, 1:2, :], AL.add)
    TT(CO[:, 0:1, :], U, P1a[:, 2:3, :], AL.add)   # c01
    U = tl("u1", 1)
    TT(U, P1b[:, 0:1, :], P1b[:, 1:2, :], AL.add)
    TT(CO[:, 2:3, :], U, P1b[:, 2:3, :], AL.add)   # c12
    U = tl("u1", 1)
    TT(U, P2[:, 0:1, :], P2[:, 1:2, :], AL.add)
    TT(CO[:, 1:2, :], U, P2[:, 2:3, :], AL.add)    # c02
    S3 = tl("I", 3)
    TT(S3, SQa, SQb, AL.add)
    CD = tl("H", 3)                                # [c00 c11 c22]
    TT(CD, S3, SQc, AL.add)

    # ---- t2 = 2*I4 = 8 c00 + c11 + c22 on PE; GT reads PSUM ------------
    t2acc = acc_psum([(CD[:, 0:1, :], eye8), CD[:, 1:2, :], CD[:, 2:3, :]])
    GT = tl("GT", 3)                               # [g0 g12 g12]
    ACT(GT[:, 0:1, :].rearrange("p n k -> p (n k)"), t2acc,
        AF.Copy, bias=16.0, scale=1.6)
    ACT(GT[:, 1:2, :].rearrange("p n k -> p (n k)"), t2acc,
        AF.Copy, bias=16.0, scale=0.2)
    ACT(GT[:, 2:3, :].rearrange("p n k -> p (n k)"), t2acc,
        AF.Copy, bias=16.0, scale=0.2)

    # ---- A = cof(C): LLd = [a00 a11 a22], LLo = [a01 a02 a12] ----------
    SQC = tl(sqc_tag, 3)
    ACT(SQC, CO, AF.Square)                        # [q01 q02 q12]
    LL = tl("LL", 6)
    MID = tl("A", 3)                               # [c11c22 c00c22 c00c11]
    TT(MID[:, 2::-2, :], CD[:, 0:2, :], CD[:, 1:3, :], AL.mult)
    TT(MID[:, 1:2, :], CD[:, 0:1, :], CD[:, 2:3, :], AL.mult)
    TT(LL[:, 0:3, :], MID, SQC[:, ::-1, :], AL.subtract)
    MB = tl("B", 3)                                # [c02c12 c01c12 c01c02]
    TT(MB[:, 2::-2, :], CO[:, 0:2, :], CO[:, 1:3, :], AL.mult)
    TT(MB[:, 1:2, :], CO[:, 0:1, :], CO[:, 2:3, :], AL.mult)
    NS = tl("C", 3)                                # [c01c22 c02c11 c00c12]
    TT(NS[:, 0:1, :], CO[:, 0:1, :], CD[:, 2:3, :], AL.mult)
    TT(NS[:, 1:2, :], CO[:, 1:2, :], CD[:, 1:2, :], AL.mult)
    TT(NS[:, 2:3, :], CD[:, 0:1, :], CO[:, 2:3, :], AL.mult)
    TT(LL[:, 3:6, :], MB, NS, AL.subtract)

    # ---- I3 = det C on PE; ln reads PSUM; r3 = exp(-ln I3) -------------
    PI2 = tl("t2a", 2)
    TT(PI2, CO[:, 0:2, :], LL[:, 3:5, :], AL.mult)  # c01a01 c02a02
    PI1 = tl("u2", 1)
    TT(PI1, CD[:, 0:1, :], LL[:, 0:1, :], AL.mult)  # c00a00
    i3acc = acc_psum([PI1, PI2[:, 0:1, :], PI2[:, 1:2, :]])
    LN3 = tl("u4", 1)
    ACT(LN3.rearrange("p n k -> p (n k)"), i3acc, AF.Ln)
    R3 = tl("u3", 1)
    ACT(R3, LN3, AF.Exp, scale=-1.0)

    # ---- t3 = 2*I5 = 8 a00 + a11 + a22 on PE ---------------------------
    E8A = tl("u5", 1)
    ACT(E8A, LL[:, 0:1, :], AF.Copy, scale=8.0)    # 8 a00 (for That products)
    t3acc = acc_psum([(LL[:, 0:1, :], eye8), LL[:, 1:2, :], LL[:, 2:3, :]])

    # ---- per-sample scalars (SQ3/t3m read t3 PSUM directly) ------------
    SQ3 = tl("u2", 1)
    ACT(SQ3.rearrange("p n k -> p (n k)"), t3acc, AF.Square, scale=SQRT02)
    XLs = tl("t2a", 2)
    ACT(XLs[:, 0:1, :], SQ3, AF.Copy, bias=-56.0)
    ACT(XLs[:, 1:2, :].rearrange("p n k -> p (n k)"), t3acc,
        AF.Copy, scale=-0.2)
    XL = tl("XL", 2)                               # (xkr, lamm)
    TT(XL, XLs, bc(R3, 2), AL.mult)
    KT = tl("u6", 1)                               # xk20 on DVE (4x mode):
    # keeps the ACT queue free of a DVE-dependent op ahead of SQ6a/SQ6b
    nc.vector.tensor_scalar_add(KT, XL[:, 0:1, :], 20.0)

    # ---- That = 2 A G A -------------------------------------------------
    SQ6a = tl("I", 3)
    ACT(SQ6a, LL[:, 0:3, :], AF.Square)            # s00 s11 s22
    SQ6b = tl("J", 3)
    ACT(SQ6b, LL[:, 3:6, :], AF.Square)            # s01 s02 s12
    # THLd on DVE: its PE-chain latency sat directly on K2d's critical path
    THLd = tl("A", 3)                              # [th00 th11 th22]
    Q80 = tl("u1", 1)
    nc.vector.tensor_scalar_mul(Q80, SQ6a[:, 0:1, :], 8.0)   # 8 s00
    Q812 = tl("t2a", 2)
    nc.vector.tensor_scalar_mul(Q812, SQ6b[:, 0:2, :], 8.0)  # 8 s01, 8 s02
    UA = tl("u4", 1)
    TT(UA, SQ6b[:, 0:1, :], SQ6b[:, 1:2, :], AL.add)
    TT(THLd[:, 0:1, :], Q80, UA, AL.add)
    UB = tl("u1", 1)
    TT(UB, SQ6a[:, 1:2, :], SQ6b[:, 2:3, :], AL.add)
    TT(THLd[:, 1:2, :], Q812[:, 0:1, :], UB, AL.add)
    UC = tl("u4", 1)
    TT(UC, SQ6b[:, 2:3, :], SQ6a[:, 2:3, :], AL.add)
    TT(THLd[:, 2:3, :], Q812[:, 1:2, :], UC, AL.add)
    E8A1 = tl("u2", 1)
    ACT(E8A1, LL[:, 3:4, :], AF.Copy, scale=8.0)   # 8 a01
    THPa = tl("E", 3)
    TT(THPa[:, 0:2, :], bc(E8A, 2), LL[:, 3:5, :], AL.mult)    # 8a00a01 8a00a02
    THPb = tl("F", 3)
    TT(THPb[:, 0:2, :], LL[:, 3:5, :], LL[:, 1:3, :], AL.mult)  # a01a11 a02a22
    THPc = tl("G", 3)
    a12b = bc(LL[:, 5:6, :], 2)
    TT(THPc[:, 1::-1, :], LL[:, 3:5, :], a12b, AL.mult)        # -> [a02a12 a01a12]
    T3P = tl(sqc_tag, 3)                           # [m812 a11a12 a22a12]
    TT(T3P[:, 0:1, :], E8A1, LL[:, 4:5, :], AL.mult)
    TT(T3P[:, 1:3, :], LL[:, 1:3, :], a12b, AL.mult)
    # off-diag sums on PE (identity-matmul accumulate) + ACT drain
    THLo = tl("B", 3)                              # [th01 th02 th12]
    for i in range(2):
        acc_chain([THPa[:, i:i + 1, :], THPb[:, i:i + 1, :],
                   THPc[:, i:i + 1, :]], THLo[:, i:i + 1, :])
    acc_chain([T3P[:, 0:1, :], T3P[:, 1:2, :], T3P[:, 2:3, :]],
              THLo[:, 2:3, :])

    # ---- S = xk20*A + lamm*That (+g on diag) ---------------------------
    lamm = XL[:, 1:2, :]
    K1d = tl("H", 3)
    TT(K1d, bc(KT, 3), LL[:, 0:3, :], AL.mult)
    K1o = tl("C", 3)
    TT(K1o, bc(KT, 3), LL[:, 3:6, :], AL.mult)
    K2d = tl("I", 3)
    TT(K2d, bc(lamm, 3), THLd, AL.mult)
    K2o = tl(sqc_tag, 3)
    TT(K2o, bc(lamm, 3), THLo, AL.mult)
    # S6 sums on DVE (~5 WU): on the PE pipeline they gated the FS waves
    S6d = tl("A", 3)                               # [S00 S11 S22]
    S6o = tl("B", 3)                               # [S01 S02 S12]
    SDt = tl("J", 3)
    TT(SDt, K1d, K2d, AL.add)
    TT(S6d, SDt, GT, AL.add)
    TT(S6o, K1o, K2o, AL.add)

    if mid_hook is not None:
        mid_hook()

    # ---- P = F S, column waves; PE accumulates, ACT drains, DMA out ----
    outv = pout_d.rearrange("p (n s) -> p n s", n=9, s=SPP)[
        :, :, t * K:(t + 1) * K]
    srow = [[(S6d, 0), (S6o, 0), (S6o, 1)],
            [(S6o, 0), (S6d, 1), (S6o, 2)],
            [(S6o, 1), (S6o, 2), (S6d, 2)]]       # srow[j][k] = S_kj
    # alternate wave tags so DVE never WAR-stalls against the lagging
    # PE accumulation pipeline reading the previous wave's products
    wtags = (("E", "F", "G"), ("H", "I", "J"))
    for j in range(3):
        wa, wb, wd = wtags[j % 2]
        ta = tl(wa, 3)
        tT, m = srow[j][0]
        TT(ta, FP[:, 0:9:3, :], bc(tT[:, m:m + 1, :], 3), AL.mult)
        tb = tl(wb, 3)
        tT, m = srow[j][1]
        TT(tb, FP[:, 1:9:3, :], bc(tT[:, m:m + 1, :], 3), AL.mult)
        td = tl(wd, 3)
        tT, m = srow[j][2]
        TT(td, FP[:, 2:9:3, :], bc(tT[:, m:m + 1, :], 3), AL.mult)
        if t == NT - 1 and j == 2:
            # final wave: sum on DVE so the kernel tail skips the PE+ACT
            # accumulate/drain pipeline; stage via a dead sp tag so the
            # write never WAR-waits on wave 1's PF drain + out-DMA (bufs=1)
            tcx = tl("GT", 3)
            TT(tcx, ta, tb, AL.add)
            pf = tl("C", 3)
            TT(pf, tcx, td, AL.add)
        else:
            pf = tl("PF", 3, pool=io, bufs=1)
            for i in range(3):
                acc_chain([ta[:, i:i + 1, :], tb[:, i:i + 1, :],
                           td[:, i:i + 1, :]], pf[:, i:i + 1, :])
        nc.sync.dma_start(outv[:, j:9:3, :], pf)


def _build():
    import concourse.bass as bass
    import concourse.tile as tile
    from concourse import bacc, mybir
    from contextlib import ExitStack

    f16 = mybir.dt.float16

    nc = bacc.Bacc("TRN2", target_bir_lowering=False, debug=False)
    fin_d = nc.dram_tensor("fin", [PART, ROW], f16, kind="ExternalInput").ap()
    eye_d = nc.dram_tensor("eye", [2, PART, PART], f16, kind="ExternalInput").ap()
    pout_d = nc.dram_tensor("pout", [PART, ROW], f16, kind="ExternalOutput").ap()

    with tile.TileContext(nc) as tc:
        with ExitStack() as ctx:
            io = ctx.enter_context(tc.tile_pool(name="io", bufs=2))
            sp = ctx.enter_context(tc.tile_pool(name="sp", bufs=1))
            ps = ctx.enter_context(
                tc.tile_pool(name="ps", bufs=1, space=bass.MemorySpace.PSUM))
            eyes = sp.tile([PART, 2, PART], f16, name="EYE", tag="EYE")
            nc.sync.dma_start(eyes, eye_d.transpose([1, 0, 2]))
            eye = eyes[:, 0, :]
            eye8 = eyes[:, 1, :]
            pre_box = [None]

            def prelude():
                import types
                f16 = mybir.dt.float16
                AFx = mybir.ActivationFunctionType
                FP = io.tile([PART, 9, K], f16, name="FPp", tag="FP", bufs=2)
                inv = fin_d.rearrange("p (n s) -> p n s", n=9, s=SPP)[
                    :, :, 1 * K:2 * K]
                nc.sync.dma_start(FP, inv)
                SQa = sp.tile([PART, 3, K], f16, name="pSQa", tag="C")
                nc.scalar.activation(SQa, FP[:, 0:3, :], AFx.Square)
                SQb = sp.tile([PART, 3, K], f16, name="pSQb", tag="D")
                nc.scalar.activation(SQb, FP[:, 3:6, :], AFx.Square)
                SQc = sp.tile([PART, 3, K], f16, name="pSQc", tag="GT")
                nc.scalar.activation(SQc, FP[:, 6:9, :], AFx.Square)
                pre_box[0] = (FP, SQa, SQb, SQc)

            _emit_tile(nc, sp, io, ps, mybir, fin_d, pout_d, eye, eye8, 0,
                       mid_hook=prelude)
            _emit_tile(nc, sp, io, ps, mybir, fin_d, pout_d, eye, eye8, 1,
                       pre=pre_box[0])

    nc.compile()
    return nc


def _get_nc():
    if "nc" not in _cache:
        _cache["nc"] = _build()
    return _cache["nc"]


def _make_in_maps(F):
    x = F.reshape(N, 9).astype(np.float16)
    eye9 = np.array([1, 0, 0, 0, 1, 0, 0, 0, 1], dtype=np.float16)
    pad = np.tile(eye9, (NPADPC - NPC, 1))
    eyes = np.stack([np.eye(PART, dtype=np.float16),
                     8.0 * np.eye(PART, dtype=np.float16)])
    in_maps = []
    for cidx in range(NCORES):
        xc = x[cidx * NPC:(cidx + 1) * NPC]
        xcp = (np.concatenate([xc, pad], axis=0)
               .reshape(PART, SPP, 9).transpose(0, 2, 1).reshape(PART, ROW))
        in_maps.append({"fin": np.ascontiguousarray(xcp), "eye": eyes})
    return in_maps


def kernel(**inputs):
    from concourse.bass_utils import run_bass_kernel_spmd

    F = np.asarray(inputs["F"], dtype=np.float32)
    nc = _get_nc()
    in_maps = _make_in_maps(F)

    res = run_bass_kernel_spmd(nc, in_maps, list(range(NCORES)))

    out = np.empty((N, 9), dtype=np.float32)
    for cidx in range(NCORES):
        oc = (np.asarray(res.results[cidx]["pout"]).astype(np.float32)
              .reshape(PART, 9, SPP).transpose(0, 2, 1).reshape(NPADPC, 9))
        out[cidx * NPC:(cidx + 1) * NPC] = oc[:NPC]
    return out.reshape(N, 3, 3)

